# revision 14
# baseline (speedup 1.0000x reference)
"""SupCon loss (nn_ConLoss) on 8 Trainium2 NeuronCores.

Math: the reference builds logits = anchor @ contrast.T with anchor rows
being label-gathered prototypes, so logits has only N_CLASSES=100 distinct
rows.  Everything factors through P' = (protos/T) @ contrast.T  [100, V*B]
(protos pre-scaled by 1/T on the host so the device works in logit units):

  per class c, column half n:  mx'[c,n] = max_j P'[c,j]
                               es[c,n]  = sum_j exp(P'[c,j] - mx'[c,n])
  per column j:                d'[j]    = P'[l_j, j]   (diagonal of logits)

The host merges per-core/per-half partials: global max m', rescaled exp-sum
E, G' = sum of d' per class (bincount -- the masked row-sum gs collapses to
a segment-sum of the diagonal since P'[c,j] with l_j==c IS d'[j]), then the
usual supcon mean-log-prob-positive in float64.

Sharding: the V*B = 8192 contrast columns split 1024 per core.  Per core:
P' = protosT @ ct on PE (bf16, fp32 PSUM), rmax on DVE (negated -> doubles
as the exp bias), exp+accumulate on ACT (the Exp activation table is
preloaded at t~1us by a dummy exp so the 1283ns table load hides under the
input DMAs), mask-multiply and the partition-axis diagonal reduce on the
otherwise idle Pool engine.  Inputs stream in 5 chunked DMAs so matmuls
pipeline with transfers; outputs leave on two parallel DMA queues (stats
via SP's HWDGE, diag via DVE's) so their fixed costs overlap.
"""

import numpy as np

import bass_rust
import concourse.bass as bass
import concourse.mybir as mybir
import concourse.tile as tile
from concourse.bass_utils import run_bass_kernel_spmd

B, V, D = 4096, 2, 512
N_CLASSES = 100
TEMPERATURE = 0.07
N_CORES = 8
CPB = (V * B) // N_CORES          # contrast columns per core = 1024
KT = D // 128                     # K-tiles of 128 = 4
PTW = KT * N_CLASSES              # packed protosT columns = 400
NH = CPB // 512                   # column halves per core = 2
HB = KT * 512                     # packed ct columns per half = 2048
N_WARM = 14                       # PE p-state warm-up matmuls


def _split_multi_waits(nc):
    """This walrus build rejects instructions carrying more than one sync
    wait.  Hoist extra waits onto same-engine NOPs inserted immediately
    before the instruction (waits execute in program order on the same
    sequencer, so semantics are unchanged)."""
    n = 0
    for f in nc.m.functions:
        for b in f.blocks:
            insts = b.instructions  # live list
            i = 0
            while i < len(insts):
                inst = insts[i]
                si = inst.sync_info
                waits = list(si.on_wait) if si and si.on_wait else []
                if len(waits) > 1:
                    inst.sync_info = bass_rust.SyncInfo(
                        on_wait=waits[-1:], on_update=list(si.on_update or [])
                    )
                    for w in waits[:-1]:
                        nop = mybir.InstNoOp(name=f"waitsplit-{n}", ins=[], outs=[])
                        n += 1
                        nop.engine = inst.engine
                        nop.sync_info = bass_rust.SyncInfo(on_wait=[w], on_update=[])
                        insts.insert(i, nop)
                        i += 1
                i += 1


_nc_cache = None


def _build_program():
    global _nc_cache
    if _nc_cache is not None:
        return _nc_cache

    f32 = mybir.dt.float32
    bf16 = mybir.dt.bfloat16
    u8 = mybir.dt.uint8
    nc = bass.Bass()
    # ct layout: [p, n*HB + a*512 + j] so each 512-col group (half n, k-tile
    # a) is a contiguous DMA-able column range.  Prefixed with protosT/T.
    # Input DMA chunks: [pt+h0a | h0b | mask | h1a | h1b] -- the per-chunk
    # transfers serialize on the DMA engines, so half-0 matmuls and stats
    # run while half 1 is still on the wire.
    ctp = nc.declare_dram_parameter("ctp", [128, PTW + NH * HB], bf16, isOutput=False)
    mask = nc.declare_dram_parameter("mask", [N_CLASSES, CPB], u8, isOutput=False)
    # outputs: diag row [1, CPB] (already /T) and per-half stats [100, 4]
    # (cols: -mx'0, -mx'1, es0, es1) -- written in place by DVE/ACT, no
    # pack/transpose stage.
    outd = nc.declare_dram_parameter("outd", [1, CPB], f32, isOutput=True)
    outs = nc.declare_dram_parameter("outs", [N_CLASSES, 2 * NH], f32, isOutput=True)

    HBH = HB // 2  # 1024 packed cols = 2 k-tiles

    with tile.TileContext(nc) as tc:
        with (
            tc.tile_pool(name="work", bufs=1) as work,
            tc.tile_pool(name="psum", bufs=1, space="PSUM") as psum,
        ):
            # --- input DMAs (SP queue, in transfer order).  The last half-1
            # k-tiles get their own small DMAs so the final PSUM accumulation
            # finishes as soon after the last ct byte as possible.
            a_t = work.tile([128, PTW + HBH], bf16, name="a_t")
            nc.sync.dma_start(out=a_t, in_=ctp[:, 0 : PTW + HBH])
            b_t = work.tile([128, HBH], bf16, name="b_t")
            nc.sync.dma_start(out=b_t, in_=ctp[:, PTW + HBH : PTW + HB])
            # mask on Pool's SWDGE queue: HWDGE holds only the 5 ct chunks,
            # so the last chunk's transfer slot isn't pushed out by
            # descriptor-generation latency (the ~1us SWDGE prep runs on the
            # otherwise idle Pool engine long before its reduces).
            mask_t = work.tile([N_CLASSES, CPB], u8)
            nc.gpsimd.dma_start(out=mask_t, in_=mask[:, :])
            c_t = work.tile([128, HBH], bf16, name="c_t")
            nc.sync.dma_start(out=c_t, in_=ctp[:, PTW + HB : PTW + HB + HBH])
            d_t = work.tile([128, 512], bf16, name="d_t")
            nc.sync.dma_start(out=d_t, in_=ctp[:, PTW + HB + HBH : PTW + HB + HBH + 512])
            e_t = work.tile([128, 512], bf16, name="e_t")
            nc.sync.dma_start(out=e_t, in_=ctp[:, PTW + HB + HBH + 512 : PTW + 2 * HB])
            pt_t = a_t[:, 0:PTW]
            # rhs source for (half n, k-tile a)
            ct_src = {
                (0, 0): a_t[:, PTW : PTW + 512],
                (0, 1): a_t[:, PTW + 512 : PTW + 1024],
                (0, 2): b_t[:, 0:512],
                (0, 3): b_t[:, 512:1024],
                (1, 0): c_t[:, 0:512],
                (1, 1): c_t[:, 512:1024],
                (1, 2): d_t[:, :],
                (1, 3): e_t[:, :],
            }

            # small init tiles (Pool memsets run in the prologue shadow)
            warm_in = work.tile([128, 64], f32)
            nc.gpsimd.memset(warm_in, 1.0)

            # Preload the Exp activation table: the first Exp pays a 1283ns
            # table load; a dummy [1,1] exp at ~1us hides it under the DMAs.
            pre_o = work.tile([1, 1], f32)
            nc.scalar.activation(
                out=pre_o,
                in_=warm_in[0:1, 0:1],
                func=mybir.ActivationFunctionType.Exp,
                scale=1.0,
            )

            p_ps = [
                psum.tile([N_CLASSES, 512], f32, name=f"pps{n}", tag=f"pps{n}")
                for n in range(NH)
            ]
            # stats cols: n -> -mx'_n, 2+n -> es_n
            stats_t = work.tile([N_CLASSES, 2 * NH], f32)
            esc = work.tile([N_CLASSES, 512], f32)   # exp scratch (reused)
            mp = work.tile([N_CLASSES, CPB], f32)    # mask * P'
            diag_t = work.tile([1, CPB], f32)        # column sums of mp

            # PE p-state primers: matmul cost is decided at dispatch time
            # from the engine's busy-streak start; keep PE chewing until the
            # real matmuls dispatch so they run at the 2.4GHz p-state.
            warm_ps = psum.tile([1, 64], f32, name="warm_ps")
            for _ in range(N_WARM):
                nc.tensor.matmul(
                    warm_ps, lhsT=warm_in[:, 0:1], rhs=warm_in[:, 0:64],
                    start=True, stop=True,
                )

            # PE: P' matmuls in DMA-arrival order
            for n in range(NH):
                for a in range(KT):
                    nc.tensor.matmul(
                        p_ps[n],
                        lhsT=pt_t[:, a * N_CLASSES : (a + 1) * N_CLASSES],
                        rhs=ct_src[(n, a)],
                        start=(a == 0),
                        stop=(a == KT - 1),
                    )

            def rmax(n):  # DVE: negated max -> exp bias and shipped stat
                nc.vector.reduce_max(
                    stats_t[:, n : n + 1], p_ps[n],
                    axis=mybir.AxisListType.X, negate=True,
                )

            def expacc(n):  # ACT: es_n = sum_j exp(P' - mx')
                nc.scalar.activation(
                    out=esc,
                    in_=p_ps[n],
                    func=mybir.ActivationFunctionType.Exp,
                    bias=stats_t[:, n : n + 1],
                    scale=1.0,
                    accum_out=stats_t[:, NH + n : NH + n + 1],
                )

            def mul(n):  # DVE: mp = mask * P'  (Pool cannot read PSUM)
                lo, hi = n * 512, (n + 1) * 512
                nc.vector.tensor_mul(mp[:, lo:hi], mask_t[:, lo:hi], p_ps[n])

            def diagc(n):  # Pool: d' = partition-sum of mp (one-hot rows)
                lo, hi = n * 512, (n + 1) * 512
                nc.gpsimd.reduce_sum(
                    diag_t[0:1, lo:hi], mp[:, lo:hi], axis=mybir.AxisListType.C
                )

            # Overlapping accesses to one tile serialize in ISSUE order (even
            # read-read), so the PSUM readers are sequenced mul -> rmax ->
            # exp: Pool's diag reduce hangs only off mul, and exp pays only
            # rmax.  Issuing diagc before rmax keeps its wait pinned to mul.
            mul(0); diagc(0); rmax(0); expacc(0)
            mul(1); diagc(1); rmax(1); expacc(1)

            # Both output DMAs on SP, diag first (it is ready earlier): its
            # HWDGE descriptor generation overlaps the tail of the exp chain,
            # and the stats DMA's SEQ+HWDGE follow immediately after.
            nc.sync.dma_start(out=outd[:, :], in_=diag_t)
            nc.sync.dma_start(out=outs[:, :], in_=stats_t)

    _split_multi_waits(nc)
    _nc_cache = nc
    return nc


def _prep_inputs(features, labels, global_protos):
    """Build the per-core input maps (shard + pack layouts on host)."""
    import ml_dtypes

    bf16 = ml_dtypes.bfloat16
    feats = np.ascontiguousarray(features, dtype=np.float32)
    protos = np.ascontiguousarray(global_protos, dtype=np.float32) / TEMPERATURE
    labels = np.asarray(labels).astype(np.int64)

    # protosT/T [D, N] packed to [128, KT*N]: pt[p, a*N+c] = protos[c, a*128+p]
    pt = (
        protos.T.reshape(KT, 128, N_CLASSES).transpose(1, 0, 2).reshape(128, -1)
    ).astype(bf16)

    in_maps = []
    bpc = B // (N_CORES // V)  # batch rows per core slab = 1024
    for k in range(N_CORES):
        b0 = bpc * (k % (N_CORES // V))
        v = k // (N_CORES // V)
        slab = feats[b0 : b0 + bpc, v, :]  # [1024, 512]
        lab = labels[b0 : b0 + bpc]
        # contrastT packed [p, n*HB + a*512 + j] (n-major halves)
        ct = (
            slab.T.reshape(KT, 128, CPB // 512, 512)
            .transpose(1, 2, 0, 3)
            .reshape(128, -1)
        ).astype(bf16)
        ctp = np.ascontiguousarray(np.concatenate([pt, ct], axis=1))
        msk = (lab[None, :] == np.arange(N_CLASSES)[:, None]).astype(np.uint8)
        in_maps.append({"ctp": ctp, "mask": np.ascontiguousarray(msk)})
    return in_maps, labels


def _combine(results, labels):
    """Merge per-core/per-half partials into the scalar loss (float64).

    Device ships, per core: outd [1, CPB] = diagonal/T, and outs [100, 4]
    with cols [-mx'0, -mx'1, es0, es1] (already in logit units, /T).
    """
    mx_a = np.stack(
        [-r["outs"][:, n].astype(np.float64) for r in results for n in range(NH)]
    )  # [16, 100]
    es_a = np.stack(
        [r["outs"][:, NH + n].astype(np.float64) for r in results for n in range(NH)]
    )
    d = np.concatenate([r["outd"][0] for r in results]).astype(np.float64)  # [8192]

    m = mx_a.max(axis=0)                                         # [100]  (= max/T)
    E = (es_a * np.exp(mx_a - m[None, :])).sum(axis=0)           # [100]
    # gs collapses to a diagonal segment-sum: P'[c,j] with l_j==c IS d'[j]
    lfull = np.tile(labels, V)                                   # [8192]
    G = np.bincount(lfull, weights=d, minlength=N_CLASSES)       # [100]  (= G/T)
    cnt = np.bincount(labels, minlength=N_CLASSES).astype(np.float64)

    mT = m[lfull]
    dT = d
    S = E[lfull] - np.exp(np.minimum(dT - mT, 0.0))
    S = np.maximum(S, 1e-300)
    npos = V * cnt[lfull] - 1.0
    numer = G[lfull] - V * cnt[lfull] * mT - (dT - mT)
    mlpp = numer / npos - np.log(S)
    return np.float32(-np.mean(mlpp))


def run(features, labels, global_protos, trace=False):
    nc = _build_program()
    in_maps, labels64 = _prep_inputs(features, labels, global_protos)
    res = run_bass_kernel_spmd(nc, in_maps, list(range(N_CORES)), trace=trace)
    loss = _combine(res.results, labels64)
    return loss, res


def kernel(features, labels, global_protos):
    loss, _ = run(features, labels, global_protos)
    return np.array(loss, dtype=np.float32)
